# revision 23
# baseline (speedup 1.0000x reference)
"""Multi-head attention (S=1024, B=8, E=1024, H=16) on 8 TRN2 NeuronCores.

Sharding: batch-parallel — core b computes batch b (all 16 heads).

Per-core device pipeline (all FLOPs on device, fp32r matmuls):
  1. v-proj:   v[j,d] = x @ Wv.T + bv           (natural layout, bias via K=1 matmul)
  2. qk-proj:  per head: qT/kT in (Dh, S) layout via W.T-permuted weights;
               bias + column-mask applied in the PSUM->SBUF copy (DVE stt).
               qe_h = [qT*m/8 ; m ; lnrT]  (66, S)   ke_h = [kT*m ; 1e9*(m-1) ; 1] (66, S)
  3. S-path:   S[i,j] = qe[0:65].T @ ke[0:65]  -> exp (ACT, accum_out=row sums)
               -> normalize (GPSIMD, 1/sden per partition) -> DMA out attn.
  4. lnr:      lnrT row = transpose(ln(1/sden)) via PE transpose, into qe row 65.
  5. S^T-path: T[j,i] = ke[0:66].T @ qe[0:66] (mask + ln-normalizer folded in)
               -> exp gives normalized P^T directly.
  6. AV:       oU[d,i] += v[j,d].T @ P^T[j,i]  accumulated over j chunks.
  7. out-proj: o[i,f] = o_norm[e,i].T @ Wo.T[e,f] + bo (bias via K=1 matmul).

The head loop is software-pipelined: head h's S^T/AV phase is emitted
interleaved with head h+1's S phase (the per-engine instruction streams are
in-order, so emission order controls overlap), and the AV matmuls for P^T
chunk jc are emitted under chunk jc+1's exp.
"""

import numpy as np

import concourse.bass as bass
import concourse.mybir as mybir
import concourse.tile as tile
from concourse import bacc
from concourse.bass import ds, ts
from concourse.bass_utils import run_bass_kernel_spmd
from concourse.masks import make_identity

S = 1024
E = 1024
H = 16
DH = 64
B = 8
NCORES = 8
F32 = mybir.dt.float32
F32R = mybir.dt.float32r
ADD = mybir.AluOpType.add
MULT = mybir.AluOpType.mult
EXP = mybir.ActivationFunctionType.Exp
LN = mybir.ActivationFunctionType.Ln


def _build():
    nc = bacc.Bacc()
    xt_d = nc.declare_dram_parameter("xt", [E, S], F32, isOutput=False)
    wqk_d = nc.declare_dram_parameter("wqk", [E, 2 * E], F32, isOutput=False)
    bqk_d = nc.declare_dram_parameter("bqk", [128, H], F32, isOutput=False)
    wv_d = nc.declare_dram_parameter("wv", [E, E], F32, isOutput=False)
    bv_d = nc.declare_dram_parameter("bv", [1, E], F32, isOutput=False)
    wo_d = nc.declare_dram_parameter("wo", [E, E], F32, isOutput=False)
    bo_d = nc.declare_dram_parameter("bo", [1, E], F32, isOutput=False)
    mrow_d = nc.declare_dram_parameter("mrow", [1, S], F32, isOutput=False)
    extk_d = nc.declare_dram_parameter("extk", [2, S], F32, isOutput=False)
    o_d = nc.declare_dram_parameter("o_out", [S, E], F32, isOutput=True)
    attn_d = nc.declare_dram_parameter("attn_out", [H, S, S], F32, isOutput=True)

    with tile.TileContext(nc) as tc:
        with tc.tile_pool(name="persist", bufs=1) as persist, tc.tile_pool(
            name="wbig", bufs=1
        ) as wbig, tc.tile_pool(name="psA", bufs=2, space="PSUM") as psA, tc.tile_pool(
            name="psBig", bufs=2, space="PSUM"
        ) as psBig, tc.tile_pool(name="psU", bufs=2, space="PSUM") as psU, tc.tile_pool(
            name="heads", bufs=2
        ) as hp, tc.tile_pool(name="pp", bufs=3) as pp, tc.tile_pool(
            name="stats", bufs=2
        ) as sp:
            # ---------- persistent tiles ----------
            xt = persist.tile([128, 8, S], F32R)
            xt_src = xt_d.ap().rearrange("(ec p) s -> p ec s", p=128).bitcast(F32R)
            for e in range(8):
                nc.sync.dma_start(out=xt[:, e, :], in_=xt_src[:, e, :])
            v_sb = persist.tile([128, 8, E], F32R)
            o_norm = persist.tile([128, 8, S], F32R)
            bq_sb = persist.tile([128, H], F32)
            nc.sync.dma_start(out=bq_sb, in_=bqk_d.ap())
            bv_sb = persist.tile([1, E], F32R)
            nc.sync.dma_start(out=bv_sb, in_=bv_d.ap().bitcast(F32R))
            bo_sb = persist.tile([1, E], F32R)
            nc.sync.dma_start(out=bo_sb, in_=bo_d.ap().bitcast(F32R))
            mr = persist.tile([1, S], F32)
            nc.sync.dma_start(out=mr, in_=mrow_d.ap())
            mbc = persist.tile([128, S], F32)
            nc.gpsimd.partition_broadcast(mbc[:, :], mr[:, :])
            ones_f32 = persist.tile([1, 128], F32)
            nc.vector.memset(ones_f32, 1.0)
            ones_row = persist.tile([1, 128], F32R)
            nc.vector.tensor_copy(ones_row[:, :], ones_f32[:, :])
            identity = persist.tile([128, 128], F32)
            make_identity(nc, identity[:, :])

            # ---------- v-proj ----------
            wv_sb = wbig.tile([128, 8, E], F32R, tag="w", name="wv_sb")
            wv_src = wv_d.ap().rearrange("(ec p) f -> p ec f", p=128).bitcast(F32R)
            def emit_vproj_unit(j, f):
                pv = psA.tile([128, 512], F32, tag="psA", name="pv")
                for e in range(8):
                    nc.tensor.matmul(
                        pv[:, :],
                        xt[:, e, ts(j, 128)],
                        wv_sb[:, e, ds(512 * f, 512)],
                        start=(e == 0),
                        stop=False,
                    )
                nc.tensor.matmul(
                    pv[:, :],
                    ones_row[:, :],
                    bv_sb[:, ds(512 * f, 512)],
                    start=False,
                    stop=True,
                )
                nc.vector.tensor_copy(v_sb[:, j, ds(512 * f, 512)], pv[:, :])

            # ---------- software-pipelined head loop ----------
            # 3-stage pipeline: during head h's tick loop we run
            #   stage A: qk-proj matmuls for head h+2 (2 per tick, spread out)
            #   stage B: S-path iteration t for head h+1 (exp + row-sum + norm)
            #   stage C: S^T + AV iteration t-1 for head h
            qe_t = [None] * H
            ke_t = [None] * H
            sden8_t = [None] * H
            rinv8_t = [None] * H
            wq_t = [None] * H
            pqk_t = [None] * H

            def emit_wqkh_dma(h):
                if h >= H:
                    return
                wqkh = hp.tile([128, 8, 128], F32R, tag="wqkh", name="wqkh", bufs=2)
                wsrc = (
                    wqk_d.ap()[:, ds(128 * h, 128)]
                    .rearrange("(ec p) c -> p ec c", p=128)
                    .bitcast(F32R)
                )
                nc.sync.dma_start(out=wqkh, in_=wsrc)
                wq_t[h] = wqkh

            def emit_proj_head_tiles(h):
                qe = hp.tile([66, S], F32R, tag="qe", name="qe", bufs=3)
                ke = hp.tile([66, S], F32R, tag="ke", name="ke", bufs=3)
                qe_t[h], ke_t[h] = qe, ke
                nc.gpsimd.dma_start(out=qe[64:65, :], in_=mrow_d.ap().bitcast(F32R))
                nc.gpsimd.dma_start(out=ke[64:66, :], in_=extk_d.ap().bitcast(F32R))
                sden8_t[h] = sp.tile([128, 8], F32, tag="sden8", name="sden8", bufs=3)
                rinv8_t[h] = sp.tile([128, 8], F32, tag="rinv8", name="rinv8", bufs=3)

            def emit_proj_mms(h, f2, epair):
                # two contraction-chunk matmuls of the f2 half for head h
                if epair == 0:
                    pqk_t[h] = psA.tile([128, 512], F32, tag="psA", name="pqk")
                pqk = pqk_t[h]
                sl = ds(512 * f2, 512)
                for e in (2 * epair, 2 * epair + 1):
                    nc.tensor.matmul(
                        pqk[:, :],
                        wq_t[h][:, e, :],
                        xt[:, e, sl],
                        start=(e == 0),
                        stop=(e == 7),
                    )

            def emit_proj_stt(h, f2):
                qe, ke, pqk = qe_t[h], ke_t[h], pqk_t[h]
                sl = ds(512 * f2, 512)
                nc.vector.scalar_tensor_tensor(
                    out=qe[0:64, sl],
                    in0=pqk[0:64, :],
                    scalar=bq_sb[0:64, h : h + 1],
                    in1=mbc[0:64, sl],
                    op0=ADD,
                    op1=MULT,
                )
                nc.vector.scalar_tensor_tensor(
                    out=ke[0:64, sl],
                    in0=pqk[64:128, :],
                    scalar=bq_sb[64:128, h : h + 1],
                    in1=mbc[64:128, sl],
                    op0=ADD,
                    op1=MULT,
                )

            def emit_proj_full(h):
                emit_proj_head_tiles(h)
                for f2 in range(2):
                    for epair in range(4):
                        emit_proj_mms(h, f2, epair)
                    emit_proj_stt(h, f2)

            def emit_s_iter(h, ic):
                qe, ke = qe_t[h], ke_t[h]
                ps = psBig.tile([128, S], F32, tag="psBig", name="ps")
                for jn in range(2):
                    jsl = ds(512 * jn, 512)
                    nc.tensor.matmul(
                        ps[:, jsl],
                        qe[0:65, ts(ic, 128)],
                        ke[0:65, jsl],
                        start=True,
                        stop=True,
                    )
                P = pp.tile([128, S], F32, tag="P", name="P", bufs=3)
                nc.scalar.activation(
                    out=P[:, :],
                    in_=ps[:, :],
                    func=EXP,
                    accum_out=sden8_t[h][:, ic : ic + 1],
                )
                nc.vector.reciprocal(
                    rinv8_t[h][:, ic : ic + 1], sden8_t[h][:, ic : ic + 1]
                )
                nc.vector.tensor_scalar_mul(
                    P[:, :], P[:, :], rinv8_t[h][:, ic : ic + 1]
                )
                nc.sync.dma_start(out=attn_d.ap()[h, ts(ic, 128), :], in_=P[:, :])

            def emit_stats(h):
                lnr8 = sp.tile([128, 8], F32, tag="lnr8", name="lnr8")
                nc.scalar.activation(out=lnr8[:, :], in_=rinv8_t[h][:, :], func=LN)
                ptr = psA.tile([8, 128], F32, tag="psA", name="ptr")
                nc.tensor.transpose(ptr[:, :], lnr8[:, :], identity[:, :])
                lnrT = sp.tile([8, 128], F32, tag="lnrT", name="lnrT")
                nc.vector.tensor_copy(lnrT[:, :], ptr[:, :])
                nc.gpsimd.dma_start(out=qe_t[h][65:66, :], in_=lnrT.bitcast(F32R))

            def emit_st_iter(h, jc, poU, PT_prev):
                qe, ke = qe_t[h], ke_t[h]
                pst = psBig.tile([128, S], F32, tag="psBig", name="pst")
                for ic2 in range(2):
                    isl = ds(512 * ic2, 512)
                    nc.tensor.matmul(
                        pst[:, isl],
                        ke[0:66, ts(jc, 128)],
                        qe[0:66, isl],
                        start=True,
                        stop=True,
                    )
                PT = pp.tile([128, S], F32R, tag="PT", name="PT", bufs=3)
                nc.scalar.activation(out=PT[:, :], in_=pst[:, :], func=EXP)
                if PT_prev is not None:
                    emit_av(h, jc - 1, poU, PT_prev)
                return PT

            def emit_av(h, jc, poU, PT):
                for ic2 in range(2):
                    nc.tensor.matmul(
                        poU[ic2][:, :],
                        v_sb[:, jc, ds(64 * h, 64)],
                        PT[:, ds(512 * ic2, 512)],
                        start=(jc == 0),
                        stop=(jc == 7),
                    )

            def emit_st_finish(h, poU):
                ec, half = h // 2, h % 2
                for ic2 in range(2):
                    nc.vector.tensor_copy(
                        o_norm[ds(64 * half, 64), ec, ds(512 * ic2, 512)],
                        poU[ic2][:, :],
                    )

            # prologue: heads 0 and 1 projected up front; head 0's S phase
            # dovetailed with the v-projection units.
            emit_wqkh_dma(0)
            emit_wqkh_dma(1)
            # wv DMAs after the first two head-weight DMAs so the first
            # qk-projection isn't queued behind 4MB of v weights.
            for e in range(8):
                nc.sync.dma_start(out=wv_sb[:, e, :], in_=wv_src[:, e, :])
            emit_proj_full(0)
            emit_wqkh_dma(2)
            emit_proj_full(1)
            for ic in range(8):
                emit_s_iter(0, ic)
                emit_vproj_unit(ic, 0)
            wo_cell = [None]
            # steady state
            for h in range(H):
                emit_wqkh_dma(h + 3)
                if 1 <= h <= 8:
                    # v columns for heads 8..15 are first needed at head 8
                    emit_vproj_unit(h - 1, 1)
                    if h == 8:
                        # wo shares the wv slot; DMAs wait for last v-proj read
                        wo_sb = wbig.tile([128, 8, E], F32R, tag="w", name="wo_sb")
                        wo_src = (
                            wo_d.ap()
                            .rearrange("(ec p) f -> p ec f", p=128)
                            .bitcast(F32R)
                        )
                        for e in range(8):
                            nc.sync.dma_start(
                                out=wo_sb[:, e, :], in_=wo_src[:, e, :]
                            )
                        wo_cell[0] = wo_sb
                emit_stats(h)
                if h + 2 < H:
                    emit_proj_head_tiles(h + 2)
                poU = [
                    psU.tile([64, 512], F32, tag="poU", name=f"poU{i}")
                    for i in range(2)
                ]
                PT_prev = None
                for t in range(9):
                    if h + 2 < H and t < 8:
                        emit_proj_mms(h + 2, t // 4, t % 4)
                        if t == 3:
                            emit_proj_stt(h + 2, 0)
                        elif t == 7:
                            emit_proj_stt(h + 2, 1)
                    if h + 1 < H and t < 8:
                        emit_s_iter(h + 1, t)
                    if t >= 1:
                        PT_prev = emit_st_iter(h, t - 1, poU, PT_prev)
                emit_av(h, 7, poU, PT_prev)
                emit_st_finish(h, poU)

            # ---------- out-proj ----------
            wo_sb = wo_cell[0]
            with tc.tile_pool(name="outp", bufs=2) as op_pool:
                for ic in range(8):
                    for f in range(2):
                        po = psA.tile([128, 512], F32, tag="psA", name="po")
                        for ec2 in range(8):
                            nc.tensor.matmul(
                                po[:, :],
                                o_norm[:, ec2, ts(ic, 128)],
                                wo_sb[:, ec2, ds(512 * f, 512)],
                                start=(ec2 == 0),
                                stop=False,
                            )
                        nc.tensor.matmul(
                            po[:, :],
                            ones_row[:, :],
                            bo_sb[:, ds(512 * f, 512)],
                            start=False,
                            stop=True,
                        )
                        of = op_pool.tile([128, 512], F32, tag="of", name="of")
                        nc.vector.tensor_copy(of[:, :], po[:, :])
                        nc.sync.dma_start(
                            out=o_d.ap()[ts(ic, 128), ds(512 * f, 512)], in_=of[:, :]
                        )

    nc.compile()
    _dedupe_act_table_loads(nc)
    return nc


def _dedupe_act_table_loads(nc):
    """All activations here are Exp/Ln; both live in the
    `natural_log_exp_and_others` table set. Point every InstLoadActFuncSet at
    that set and keep only the first (entry block dominates everything),
    eliminating ~1.3us per reload on the ACT engine."""
    from concourse.hw_specs import get_activation_tables

    names = list(get_activation_tables(nc.m.arch).keys())
    target = names.index("natural_log_exp_and_others")
    first_seen = False
    for blk in nc.main_func.blocks:
        keep = []
        for inst in blk.instructions:
            if isinstance(inst, mybir.InstLoadActFuncSet):
                si = getattr(inst, "sync_info", None)
                has_sync = si is not None and (
                    len(si.on_wait) > 0 or len(si.on_update) > 0
                )
                if not first_seen or has_sync:
                    inst.act_func_set_id = target
                    first_seen = True
                    keep.append(inst)
                # else drop the redundant load
            else:
                keep.append(inst)
        blk.instructions[:] = keep


_NC = None


def _get_nc():
    global _NC
    if _NC is None:
        _NC = _build()
    return _NC


def _prep_inputs(x, mask, qkv_w, qkv_b, out_w, out_b):
    """Host-side shard/layout prep. Returns per-core input maps."""
    x = np.ascontiguousarray(np.asarray(x), dtype=np.float32)
    mask = np.asarray(mask)
    qkv_w = np.asarray(qkv_w, dtype=np.float32)
    qkv_b = np.asarray(qkv_b, dtype=np.float32)
    out_w = np.asarray(out_w, dtype=np.float32)
    out_b = np.asarray(out_b, dtype=np.float32)

    Wq, Wk, Wv = qkv_w[0:E], qkv_w[E : 2 * E], qkv_w[2 * E : 3 * E]
    bq, bk, bv = qkv_b[0:E], qkv_b[E : 2 * E], qkv_b[2 * E : 3 * E]

    # wqk columns per head: [Wq_h.T / 8 | Wk_h.T]
    wqk = np.empty((E, 2 * E), dtype=np.float32)
    bqk = np.empty((128, H), dtype=np.float32)
    WqT = np.ascontiguousarray(Wq.T) / 8.0
    WkT = np.ascontiguousarray(Wk.T)
    for h in range(H):
        wqk[:, 128 * h : 128 * h + 64] = WqT[:, 64 * h : 64 * h + 64]
        wqk[:, 128 * h + 64 : 128 * h + 128] = WkT[:, 64 * h : 64 * h + 64]
        bqk[0:64, h] = bq[64 * h : 64 * h + 64] / 8.0
        bqk[64:128, h] = bk[64 * h : 64 * h + 64]

    shared = {
        "wqk": np.ascontiguousarray(wqk),
        "bqk": np.ascontiguousarray(bqk),
        "wv": np.ascontiguousarray(Wv.T),
        "bv": np.ascontiguousarray(bv[None, :]),
        "wo": np.ascontiguousarray(out_w.T),
        "bo": np.ascontiguousarray(out_b[None, :]),
    }

    in_maps = []
    for b in range(B):
        m = mask[b].astype(np.float32)
        im = dict(shared)
        im["xt"] = np.ascontiguousarray(x[:, b, :].T)
        im["mrow"] = np.ascontiguousarray(m[None, :])
        im["extk"] = np.ascontiguousarray(
            np.stack([1e9 * (m - 1.0), np.ones(S, np.float32)])
        )
        in_maps.append(im)
    return in_maps


def _run(inputs, **kw):
    nc = _get_nc()
    in_maps = _prep_inputs(**inputs)
    br = run_bass_kernel_spmd(nc, in_maps, core_ids=list(range(NCORES)), **kw)
    o = np.empty((S, B, E), dtype=np.float32)
    attn = np.empty((B, H, S, S), dtype=np.float32)
    for b in range(B):
        o[:, b, :] = br.results[b]["o_out"]
        attn[b] = br.results[b]["attn_out"]
    return (o, attn), br


def kernel(x, mask, qkv_w, qkv_b, out_w, out_b):
    out, _ = _run(
        dict(x=x, mask=mask, qkv_w=qkv_w, qkv_b=qkv_b, out_w=out_w, out_b=out_b)
    )
    return out


# revision 25
# speedup vs baseline: 1.0156x; 1.0156x over previous
"""Multi-head attention (S=1024, B=8, E=1024, H=16) on 8 TRN2 NeuronCores.

Sharding: batch-parallel — core b computes batch b (all 16 heads).

Per-core device pipeline (all FLOPs on device, fp32r matmuls):
  1. v-proj:   v[j,d] = x @ Wv.T + bv           (natural layout, bias via K=1 matmul)
  2. qk-proj:  per head: qT/kT in (Dh, S) layout via W.T-permuted weights;
               bias + column-mask applied in the PSUM->SBUF copy (DVE stt).
               qe_h = [qT*m/8 ; m ; lnrT]  (66, S)   ke_h = [kT*m ; 1e9*(m-1) ; 1] (66, S)
  3. S-path:   S[i,j] = qe[0:65].T @ ke[0:65]  -> exp (ACT, accum_out=row sums)
               -> normalize (GPSIMD, 1/sden per partition) -> DMA out attn.
  4. lnr:      lnrT row = transpose(ln(1/sden)) via PE transpose, into qe row 65.
  5. S^T-path: T[j,i] = ke[0:66].T @ qe[0:66] (mask + ln-normalizer folded in)
               -> exp gives normalized P^T directly.
  6. AV:       oU[d,i] += v[j,d].T @ P^T[j,i]  accumulated over j chunks.
  7. out-proj: o[i,f] = o_norm[e,i].T @ Wo.T[e,f] + bo (bias via K=1 matmul).

The head loop is software-pipelined: head h's S^T/AV phase is emitted
interleaved with head h+1's S phase (the per-engine instruction streams are
in-order, so emission order controls overlap), and the AV matmuls for P^T
chunk jc are emitted under chunk jc+1's exp.
"""

import numpy as np

import concourse.bass as bass
import concourse.mybir as mybir
import concourse.tile as tile
from concourse import bacc
from concourse.bass import ds, ts
from concourse.bass_utils import run_bass_kernel_spmd
from concourse.masks import make_identity

S = 1024
E = 1024
H = 16
DH = 64
B = 8
NCORES = 8
F32 = mybir.dt.float32
F32R = mybir.dt.float32r
ADD = mybir.AluOpType.add
MULT = mybir.AluOpType.mult
EXP = mybir.ActivationFunctionType.Exp
LN = mybir.ActivationFunctionType.Ln


def _build():
    nc = bacc.Bacc()
    xt_d = nc.declare_dram_parameter("xt", [E, S], F32, isOutput=False)
    wqk_d = nc.declare_dram_parameter("wqk", [E, 2 * E], F32, isOutput=False)
    bqk_d = nc.declare_dram_parameter("bqk", [128, H], F32, isOutput=False)
    wv_d = nc.declare_dram_parameter("wv", [E, E], F32, isOutput=False)
    bv_d = nc.declare_dram_parameter("bv", [1, E], F32, isOutput=False)
    wo_d = nc.declare_dram_parameter("wo", [E, E], F32, isOutput=False)
    bo_d = nc.declare_dram_parameter("bo", [1, E], F32, isOutput=False)
    mrow_d = nc.declare_dram_parameter("mrow", [1, S], F32, isOutput=False)
    extk_d = nc.declare_dram_parameter("extk", [2, S], F32, isOutput=False)
    o_d = nc.declare_dram_parameter("o_out", [S, E], F32, isOutput=True)
    attn_d = nc.declare_dram_parameter("attn_out", [H, S, S], F32, isOutput=True)

    with tile.TileContext(nc) as tc:
        with tc.tile_pool(name="persist", bufs=1) as persist, tc.tile_pool(
            name="wbig", bufs=1
        ) as wbig, tc.tile_pool(name="psA", bufs=2, space="PSUM") as psA, tc.tile_pool(
            name="psBig", bufs=2, space="PSUM"
        ) as psBig, tc.tile_pool(name="psU", bufs=2, space="PSUM") as psU, tc.tile_pool(
            name="heads", bufs=2
        ) as hp, tc.tile_pool(name="pp", bufs=3) as pp, tc.tile_pool(
            name="stats", bufs=2
        ) as sp:
            # ---------- persistent tiles ----------
            xt = persist.tile([128, 8, S], F32R)
            xt_src = xt_d.ap().rearrange("(ec p) s -> p ec s", p=128).bitcast(F32R)
            for e in range(8):
                nc.sync.dma_start(out=xt[:, e, :], in_=xt_src[:, e, :])
            v_sb = persist.tile([128, 8, E], F32R)
            o_norm = persist.tile([128, 8, S], F32R)
            bq_sb = persist.tile([128, H], F32)
            nc.sync.dma_start(out=bq_sb, in_=bqk_d.ap())
            bv_sb = persist.tile([1, E], F32R)
            nc.sync.dma_start(out=bv_sb, in_=bv_d.ap().bitcast(F32R))
            bo_sb = persist.tile([1, E], F32R)
            nc.sync.dma_start(out=bo_sb, in_=bo_d.ap().bitcast(F32R))
            mr = persist.tile([1, S], F32)
            nc.sync.dma_start(out=mr, in_=mrow_d.ap())
            mbc = persist.tile([128, S], F32)
            nc.gpsimd.partition_broadcast(mbc[:, :], mr[:, :])
            ones_f32 = persist.tile([1, 128], F32)
            nc.vector.memset(ones_f32, 1.0)
            ones_row = persist.tile([1, 128], F32R)
            nc.vector.tensor_copy(ones_row[:, :], ones_f32[:, :])
            identity = persist.tile([128, 128], F32)
            make_identity(nc, identity[:, :])

            # ---------- v-proj ----------
            wv_sb = wbig.tile([128, 8, E], F32R, tag="w", name="wv_sb")
            wv_src = wv_d.ap().rearrange("(ec p) f -> p ec f", p=128).bitcast(F32R)
            def emit_vproj_unit(j, f):
                pv = psA.tile([128, 512], F32, tag="psA", name="pv")
                for e in range(8):
                    nc.tensor.matmul(
                        pv[:, :],
                        xt[:, e, ts(j, 128)],
                        wv_sb[:, e, ds(512 * f, 512)],
                        start=(e == 0),
                        stop=False,
                    )
                nc.tensor.matmul(
                    pv[:, :],
                    ones_row[:, :],
                    bv_sb[:, ds(512 * f, 512)],
                    start=False,
                    stop=True,
                )
                nc.vector.tensor_copy(v_sb[:, j, ds(512 * f, 512)], pv[:, :])

            # ---------- software-pipelined head loop ----------
            # 3-stage pipeline: during head h's tick loop we run
            #   stage A: qk-proj matmuls for head h+2 (2 per tick, spread out)
            #   stage B: S-path iteration t for head h+1 (exp + row-sum + norm)
            #   stage C: S^T + AV iteration t-1 for head h
            qe_t = [None] * H
            ke_t = [None] * H
            sden8_t = [None] * H
            rinv8_t = [None] * H
            wq_t = [None] * H
            pqk_t = [None] * H

            def emit_wqkh_dma(h):
                if h >= H:
                    return
                wqkh = hp.tile([128, 8, 128], F32R, tag="wqkh", name="wqkh", bufs=2)
                wsrc = (
                    wqk_d.ap()[:, ds(128 * h, 128)]
                    .rearrange("(ec p) c -> p ec c", p=128)
                    .bitcast(F32R)
                )
                nc.sync.dma_start(out=wqkh, in_=wsrc)
                wq_t[h] = wqkh

            def emit_proj_head_tiles(h):
                qe = hp.tile([66, S], F32R, tag="qe", name="qe", bufs=3)
                ke = hp.tile([66, S], F32R, tag="ke", name="ke", bufs=3)
                qe_t[h], ke_t[h] = qe, ke
                nc.gpsimd.dma_start(out=qe[64:65, :], in_=mrow_d.ap().bitcast(F32R))
                nc.gpsimd.dma_start(out=ke[64:66, :], in_=extk_d.ap().bitcast(F32R))
                sden8_t[h] = sp.tile([128, 8], F32, tag="sden8", name="sden8", bufs=3)
                rinv8_t[h] = sp.tile([128, 8], F32, tag="rinv8", name="rinv8", bufs=3)

            def emit_proj_mms(h, f2, epair):
                # two contraction-chunk matmuls of the f2 half for head h
                if epair == 0:
                    pqk_t[h] = psA.tile([128, 512], F32, tag="psA", name="pqk")
                pqk = pqk_t[h]
                sl = ds(512 * f2, 512)
                for e in (2 * epair, 2 * epair + 1):
                    nc.tensor.matmul(
                        pqk[:, :],
                        wq_t[h][:, e, :],
                        xt[:, e, sl],
                        start=(e == 0),
                        stop=(e == 7),
                    )

            def emit_proj_stt(h, f2):
                qe, ke, pqk = qe_t[h], ke_t[h], pqk_t[h]
                sl = ds(512 * f2, 512)
                nc.vector.scalar_tensor_tensor(
                    out=qe[0:64, sl],
                    in0=pqk[0:64, :],
                    scalar=bq_sb[0:64, h : h + 1],
                    in1=mbc[0:64, sl],
                    op0=ADD,
                    op1=MULT,
                )
                nc.vector.scalar_tensor_tensor(
                    out=ke[0:64, sl],
                    in0=pqk[64:128, :],
                    scalar=bq_sb[64:128, h : h + 1],
                    in1=mbc[64:128, sl],
                    op0=ADD,
                    op1=MULT,
                )

            def emit_proj_full(h):
                emit_proj_head_tiles(h)
                for f2 in range(2):
                    for epair in range(4):
                        emit_proj_mms(h, f2, epair)
                    emit_proj_stt(h, f2)

            def emit_s_iter(h, ic):
                qe, ke = qe_t[h], ke_t[h]
                ps = psBig.tile([128, S], F32, tag="psBig", name="ps")
                for jn in range(2):
                    jsl = ds(512 * jn, 512)
                    nc.tensor.matmul(
                        ps[:, jsl],
                        qe[0:65, ts(ic, 128)],
                        ke[0:65, jsl],
                        start=True,
                        stop=True,
                    )
                P = pp.tile([128, S], F32, tag="P", name="P", bufs=3)
                nc.scalar.activation(
                    out=P[:, :],
                    in_=ps[:, :],
                    func=EXP,
                    accum_out=sden8_t[h][:, ic : ic + 1],
                )
                nc.vector.reciprocal(
                    rinv8_t[h][:, ic : ic + 1], sden8_t[h][:, ic : ic + 1]
                )
                nc.vector.tensor_scalar_mul(
                    P[:, :], P[:, :], rinv8_t[h][:, ic : ic + 1]
                )
                nc.sync.dma_start(out=attn_d.ap()[h, ts(ic, 128), :], in_=P[:, :])

            def emit_stats(h):
                lnr8 = sp.tile([128, 8], F32, tag="lnr8", name="lnr8")
                nc.scalar.activation(out=lnr8[:, :], in_=rinv8_t[h][:, :], func=LN)
                ptr = psA.tile([8, 128], F32, tag="psA", name="ptr")
                nc.tensor.transpose(ptr[:, :], lnr8[:, :], identity[:, :])
                lnrT = sp.tile([8, 128], F32, tag="lnrT", name="lnrT")
                nc.vector.tensor_copy(lnrT[:, :], ptr[:, :])
                nc.gpsimd.dma_start(out=qe_t[h][65:66, :], in_=lnrT.bitcast(F32R))

            def emit_st_iter(h, jc, poU, PT_prev):
                qe, ke = qe_t[h], ke_t[h]
                pst = psBig.tile([128, S], F32, tag="psBig", name="pst")
                for ic2 in range(2):
                    isl = ds(512 * ic2, 512)
                    nc.tensor.matmul(
                        pst[:, isl],
                        ke[0:66, ts(jc, 128)],
                        qe[0:66, isl],
                        start=True,
                        stop=True,
                    )
                PT = pp.tile([128, S], F32R, tag="PT", name="PT", bufs=3)
                nc.scalar.activation(out=PT[:, :], in_=pst[:, :], func=EXP)
                if PT_prev is not None:
                    emit_av(h, jc - 1, poU, PT_prev)
                return PT

            def emit_av(h, jc, poU, PT):
                for ic2 in range(2):
                    nc.tensor.matmul(
                        poU[ic2][:, :],
                        v_sb[:, jc, ds(64 * h, 64)],
                        PT[:, ds(512 * ic2, 512)],
                        start=(jc == 0),
                        stop=(jc == 7),
                    )

            def emit_st_finish(h, poU):
                ec, half = h // 2, h % 2
                for ic2 in range(2):
                    nc.vector.tensor_copy(
                        o_norm[ds(64 * half, 64), ec, ds(512 * ic2, 512)],
                        poU[ic2][:, :],
                    )

            # prologue: heads 0 and 1 projected up front; head 0's S phase
            # dovetailed with the v-projection units.
            emit_wqkh_dma(0)
            emit_wqkh_dma(1)
            # wv DMAs after the first two head-weight DMAs so the first
            # qk-projection isn't queued behind 4MB of v weights.
            for e in range(8):
                nc.sync.dma_start(out=wv_sb[:, e, :], in_=wv_src[:, e, :])
            emit_proj_full(0)
            emit_wqkh_dma(2)
            emit_proj_full(1)
            for ic in range(8):
                emit_s_iter(0, ic)
                emit_vproj_unit(ic, 0)
            wo_cell = [None]
            # steady state
            for h in range(H):
                emit_wqkh_dma(h + 3)
                if 1 <= h <= 8:
                    # v columns for heads 8..15 are first needed at head 8
                    emit_vproj_unit(h - 1, 1)
                    if h == 8:
                        # wo shares the wv slot; DMAs wait for last v-proj read
                        wo_cell[0] = wbig.tile(
                            [128, 8, E], F32R, tag="w", name="wo_sb"
                        )
                if 8 <= h <= 11:
                    # spread the 4MB wo load over four heads
                    wo_src = (
                        wo_d.ap().rearrange("(ec p) f -> p ec f", p=128).bitcast(F32R)
                    )
                    for e in (2 * (h - 8), 2 * (h - 8) + 1):
                        nc.sync.dma_start(
                            out=wo_cell[0][:, e, :], in_=wo_src[:, e, :]
                        )
                emit_stats(h)
                if h + 2 < H:
                    emit_proj_head_tiles(h + 2)
                poU = [
                    psU.tile([64, 512], F32, tag="poU", name=f"poU{i}")
                    for i in range(2)
                ]
                PT_prev = None
                for t in range(9):
                    if h + 2 < H and t < 8:
                        emit_proj_mms(h + 2, t // 4, t % 4)
                        if t == 3:
                            emit_proj_stt(h + 2, 0)
                        elif t == 7:
                            emit_proj_stt(h + 2, 1)
                    if h + 1 < H and t < 8:
                        emit_s_iter(h + 1, t)
                    if t >= 1:
                        PT_prev = emit_st_iter(h, t - 1, poU, PT_prev)
                emit_av(h, 7, poU, PT_prev)
                emit_st_finish(h, poU)

            # ---------- out-proj ----------
            wo_sb = wo_cell[0]
            with tc.tile_pool(name="outp", bufs=2) as op_pool:
                for ic in range(8):
                    for f in range(2):
                        po = psA.tile([128, 512], F32, tag="psA", name="po")
                        for ec2 in range(8):
                            nc.tensor.matmul(
                                po[:, :],
                                o_norm[:, ec2, ts(ic, 128)],
                                wo_sb[:, ec2, ds(512 * f, 512)],
                                start=(ec2 == 0),
                                stop=False,
                            )
                        nc.tensor.matmul(
                            po[:, :],
                            ones_row[:, :],
                            bo_sb[:, ds(512 * f, 512)],
                            start=False,
                            stop=True,
                        )
                        of = op_pool.tile([128, 512], F32, tag="of", name="of")
                        nc.vector.tensor_copy(of[:, :], po[:, :])
                        nc.sync.dma_start(
                            out=o_d.ap()[ts(ic, 128), ds(512 * f, 512)], in_=of[:, :]
                        )

    nc.compile()
    _dedupe_act_table_loads(nc)
    return nc


def _dedupe_act_table_loads(nc):
    """All activations here are Exp/Ln; both live in the
    `natural_log_exp_and_others` table set. Point every InstLoadActFuncSet at
    that set and keep only the first (entry block dominates everything),
    eliminating ~1.3us per reload on the ACT engine."""
    from concourse.hw_specs import get_activation_tables

    names = list(get_activation_tables(nc.m.arch).keys())
    target = names.index("natural_log_exp_and_others")
    first_seen = False
    for blk in nc.main_func.blocks:
        keep = []
        for inst in blk.instructions:
            if isinstance(inst, mybir.InstLoadActFuncSet):
                si = getattr(inst, "sync_info", None)
                has_sync = si is not None and (
                    len(si.on_wait) > 0 or len(si.on_update) > 0
                )
                if not first_seen or has_sync:
                    inst.act_func_set_id = target
                    first_seen = True
                    keep.append(inst)
                # else drop the redundant load
            else:
                keep.append(inst)
        blk.instructions[:] = keep


_NC = None


def _get_nc():
    global _NC
    if _NC is None:
        _NC = _build()
    return _NC


def _prep_inputs(x, mask, qkv_w, qkv_b, out_w, out_b):
    """Host-side shard/layout prep. Returns per-core input maps."""
    x = np.ascontiguousarray(np.asarray(x), dtype=np.float32)
    mask = np.asarray(mask)
    qkv_w = np.asarray(qkv_w, dtype=np.float32)
    qkv_b = np.asarray(qkv_b, dtype=np.float32)
    out_w = np.asarray(out_w, dtype=np.float32)
    out_b = np.asarray(out_b, dtype=np.float32)

    Wq, Wk, Wv = qkv_w[0:E], qkv_w[E : 2 * E], qkv_w[2 * E : 3 * E]
    bq, bk, bv = qkv_b[0:E], qkv_b[E : 2 * E], qkv_b[2 * E : 3 * E]

    # wqk columns per head: [Wq_h.T / 8 | Wk_h.T]
    wqk = np.empty((E, 2 * E), dtype=np.float32)
    bqk = np.empty((128, H), dtype=np.float32)
    WqT = np.ascontiguousarray(Wq.T) / 8.0
    WkT = np.ascontiguousarray(Wk.T)
    for h in range(H):
        wqk[:, 128 * h : 128 * h + 64] = WqT[:, 64 * h : 64 * h + 64]
        wqk[:, 128 * h + 64 : 128 * h + 128] = WkT[:, 64 * h : 64 * h + 64]
        bqk[0:64, h] = bq[64 * h : 64 * h + 64] / 8.0
        bqk[64:128, h] = bk[64 * h : 64 * h + 64]

    shared = {
        "wqk": np.ascontiguousarray(wqk),
        "bqk": np.ascontiguousarray(bqk),
        "wv": np.ascontiguousarray(Wv.T),
        "bv": np.ascontiguousarray(bv[None, :]),
        "wo": np.ascontiguousarray(out_w.T),
        "bo": np.ascontiguousarray(out_b[None, :]),
    }

    in_maps = []
    for b in range(B):
        m = mask[b].astype(np.float32)
        im = dict(shared)
        im["xt"] = np.ascontiguousarray(x[:, b, :].T)
        im["mrow"] = np.ascontiguousarray(m[None, :])
        im["extk"] = np.ascontiguousarray(
            np.stack([1e9 * (m - 1.0), np.ones(S, np.float32)])
        )
        in_maps.append(im)
    return in_maps


def _run(inputs, **kw):
    nc = _get_nc()
    in_maps = _prep_inputs(**inputs)
    br = run_bass_kernel_spmd(nc, in_maps, core_ids=list(range(NCORES)), **kw)
    o = np.empty((S, B, E), dtype=np.float32)
    attn = np.empty((B, H, S, S), dtype=np.float32)
    for b in range(B):
        o[:, b, :] = br.results[b]["o_out"]
        attn[b] = br.results[b]["attn_out"]
    return (o, attn), br


def kernel(x, mask, qkv_w, qkv_b, out_w, out_b):
    out, _ = _run(
        dict(x=x, mask=mask, qkv_w=qkv_w, qkv_b=qkv_b, out_w=out_w, out_b=out_b)
    )
    return out
